# revision 1
# baseline (speedup 1.0000x reference)
"""Grouped self-attention (B=2, S=2048, D=1024, H=16, hd=64) on 8 trn2 cores.

Sharding: core c = b*4 + g handles batch b, heads [4g, 4g+4).

Key simplification: the reference's RoPE indexes its cos/sin cache by the
BATCH dim and uses neg_half = [t_first, -t_second], so rope(t)[b,s,h,d] =
t * (cos(b*th[d%32]) + sign(d)*sin(b*th[d%32])) — a pure per-(b,d) scale
that folds into rows of Wq/Wk on the host. The device kernel is then just
QKV projection + softmax attention.

Device layout per core:
  xt    [8,128,2048]  = x[b].T          (k-major chunks)
  wqt   [8,128,256]   = (Wq*ceff).T cols for this core's 4 heads
  wkt   [8,128,256]   = (Wk*ceff).T cols
  wvt   [8,128,256]   = Wv.T cols
  maskb [128,16]      = (mask[b]-1)*3e4 as [S] -> [16,128].T
  out   [2048,256]

QT/KT stored [128(2 heads x 64d), 2048 s]; scores computed transposed
[k part, q free] so softmax denominator comes from an extra ones-column in
the V stationary operand; exp on ACT with fused 1/8 scale + mask bias; PV
as outT = V_aug.T @ expT accumulated over k-tiles; final PE transpose +
reciprocal-normalize; all matmuls in float32r (full PE rate, N>=256).
"""

import os
import numpy as np
from contextlib import ExitStack

import concourse.bass as bass
import concourse.bacc as bacc
import concourse.tile as tile
from concourse import mybir
from concourse.bass_utils import run_bass_kernel_spmd
from concourse.masks import make_identity

F32 = mybir.dt.float32
F32R = mybir.dt.float32r
EXP = mybir.ActivationFunctionType.Exp

B, S, D, H, HD = 2, 2048, 1024, 16, 64
NCORES = 8
HPC = H // 4  # 4 heads per core

_CACHE = {}


def _build_nc():
    nc = bacc.Bacc("TRN2", target_bir_lowering=False, debug=False)
    xt_d = nc.declare_dram_parameter("xt", [8, 128, S], F32, isOutput=False)
    wqt_d = nc.declare_dram_parameter("wqt", [8, 128, 256], F32, isOutput=False)
    wkt_d = nc.declare_dram_parameter("wkt", [8, 128, 256], F32, isOutput=False)
    wvt_d = nc.declare_dram_parameter("wvt", [8, 128, 256], F32, isOutput=False)
    mb_d = nc.declare_dram_parameter("maskb", [128, 16], F32, isOutput=False)
    out_d = nc.declare_dram_parameter("out", [S, 256], F32, isOutput=True)

    with tile.TileContext(nc) as tc, ExitStack() as ctx:
        const = ctx.enter_context(tc.tile_pool(name="const", bufs=1))
        xpool = ctx.enter_context(tc.tile_pool(name="x", bufs=1))
        wpool = ctx.enter_context(tc.tile_pool(name="w", bufs=1))
        qkpool = ctx.enter_context(tc.tile_pool(name="qk", bufs=1))
        vpool = ctx.enter_context(tc.tile_pool(name="v", bufs=1))
        ostage = ctx.enter_context(tc.tile_pool(name="ostage", bufs=2))
        osb = ctx.enter_context(tc.tile_pool(name="osb", bufs=2))
        epool = ctx.enter_context(tc.tile_pool(name="et", bufs=3))
        small = ctx.enter_context(tc.tile_pool(name="small", bufs=4))
        scp = ctx.enter_context(tc.tile_pool(name="scp", bufs=2, space="PSUM"))
        pvp = ctx.enter_context(tc.tile_pool(name="pvp", bufs=2, space="PSUM"))
        tpp = ctx.enter_context(tc.tile_pool(name="tpp", bufs=2, space="PSUM"))

        ident = const.tile([128, 128], F32)
        make_identity(nc, ident)
        mb = const.tile([128, 16], F32)
        nc.sync.dma_start(mb[:], mb_d[:])

        xt = xpool.tile([128, 8 * S], F32R)
        for c in range(8):
            nc.sync.dma_start(xt[:, c * S:(c + 1) * S], xt_d[c].bitcast(F32R))
        wq = wpool.tile([128, 8 * 256], F32R, tag="wq")
        wk = wpool.tile([128, 8 * 256], F32R, tag="wk")
        wv = wpool.tile([128, 8 * 256], F32R, tag="wv")
        for wtile, wd in ((wq, wqt_d), (wk, wkt_d), (wv, wvt_d)):
            for c in range(8):
                nc.sync.dma_start(wtile[:, c * 256:(c + 1) * 256], wd[c].bitcast(F32R))

        # ---- phase 1: projections ----
        # QT/KT per 2-head pair: [128 (2h x 64d), 2048 s]
        qt = [qkpool.tile([128, S], F32R, tag=f"qt{p}", name=f"qt{p}") for p in range(2)]
        kt = [qkpool.tile([128, S], F32R, tag=f"kt{p}", name=f"kt{p}") for p in range(2)]
        for pair in range(2):
            for wtile, dst in ((wq, qt[pair]), (wk, kt[pair])):
                for nb in range(4):
                    ps = scp.tile([128, 512], F32, tag="sc")
                    for kc in range(8):
                        lo = kc * 256 + pair * 128
                        nc.tensor.matmul(
                            ps[:],
                            lhsT=wtile[:, lo:lo + 128],
                            rhs=xt[:, kc * S + nb * 512: kc * S + nb * 512 + 512],
                            start=(kc == 0), stop=(kc == 7))
                    if nb % 2 == 0:
                        nc.vector.tensor_copy(dst[:, nb * 512:(nb + 1) * 512], ps[:])
                    else:
                        nc.scalar.copy(dst[:, nb * 512:(nb + 1) * 512], ps[:])

        # V (all 4 heads): natural layout [s part tiles, 4h x 65] with ones col
        v_sb = vpool.tile([128, 16, 4, 65], F32R)
        nc.vector.memset(v_sb[:, :, :, 64:65].bitcast(F32), 1.0)
        for m in range(16):
            pv = scp.tile([128, 256], F32, tag="sc")
            for kc in range(8):
                nc.tensor.matmul(
                    pv[:],
                    lhsT=xt[:, kc * S + m * 128: kc * S + m * 128 + 128],
                    rhs=wv[:, kc * 256:(kc + 1) * 256],
                    start=(kc == 0), stop=(kc == 7))
            for h in range(4):
                if h % 2 == 0:
                    nc.vector.tensor_copy(v_sb[:, m, h, 0:64], pv[:, h * 64:(h + 1) * 64])
                else:
                    nc.scalar.copy(v_sb[:, m, h, 0:64], pv[:, h * 64:(h + 1) * 64])

        # ---- phase 2: attention ----
        for pair in range(2):
            ost = ostage.tile([128, 16 * 128], F32)
            for hh in range(2):
                h = pair * 2 + hh
                hoff = hh * 64
                for qh in range(2):
                    pv0 = pvp.tile([65, 512], F32, tag="pv")
                    pv1 = pvp.tile([65, 512], F32, tag="pv")
                    pvs = (pv0, pv1)
                    for kb in range(16):
                        ps = scp.tile([128, 1024], F32, tag="sc")
                        for j in range(2):
                            q0 = qh * 1024 + j * 512
                            nc.tensor.matmul(
                                ps[:, j * 512:(j + 1) * 512],
                                lhsT=kt[pair][hoff:hoff + 64, kb * 128:(kb + 1) * 128],
                                rhs=qt[pair][hoff:hoff + 64, q0:q0 + 512],
                                start=True, stop=True)
                        et = epool.tile([128, 1024], F32R)
                        nc.scalar.activation(et[:], ps[:], EXP,
                                             bias=mb[:, kb:kb + 1], scale=0.125)
                        for j in range(2):
                            nc.tensor.matmul(
                                pvs[j][:],
                                lhsT=v_sb[:, kb, h, :],
                                rhs=et[:, j * 512:(j + 1) * 512],
                                start=(kb == 0), stop=(kb == 15))
                    for j in range(2):
                        ot = osb.tile([65, 512], F32, tag="ot")
                        nc.vector.tensor_copy(ot[:], pvs[j][:])
                        for t in range(4):
                            tp = tpp.tile([128, 65], F32, tag="tp")
                            nc.tensor.transpose(tp[:], ot[:, t * 128:(t + 1) * 128],
                                                ident[0:65, 0:65])
                            rc = small.tile([128, 1], F32, tag="rc")
                            nc.vector.reciprocal(rc[:], tp[:, 64:65])
                            mi = qh * 8 + j * 4 + t
                            nc.vector.tensor_scalar_mul(
                                ost[:, mi * 128 + hoff: mi * 128 + hoff + 64],
                                tp[:, 0:64], rc[:])
            for m in range(16):
                nc.sync.dma_start(out_d[m * 128:(m + 1) * 128, pair * 128:(pair + 1) * 128],
                                  ost[:, m * 128:(m + 1) * 128])
    nc.compile()
    return nc


def _host_prep(x, attention_mask, Wq, Wk, Wv):
    x = np.asarray(x, dtype=np.float32)
    mask = np.asarray(attention_mask)
    Wq = np.asarray(Wq, dtype=np.float32)
    Wk = np.asarray(Wk, dtype=np.float32)
    Wv = np.asarray(Wv, dtype=np.float32)

    # rope fold: c_eff[b, d] = cos(b*th[d%32]) + sign(d)*sin(b*th[d%32])
    j = np.arange(0, HD, 2, dtype=np.float64) / HD          # [32]
    theta = 1.0 / (10000.0 ** j)                            # [32]
    dd = np.arange(HD)
    sign = np.where(dd < 32, 1.0, -1.0)
    in_maps = []
    wvt_full = np.ascontiguousarray(Wv.T)                   # [1024,1024]
    for b in range(B):
        ang = b * theta                                     # [32]
        ce = np.cos(ang[dd % 32]) + sign * np.sin(ang[dd % 32])  # [64]
        ccol = np.tile(ce, H).astype(np.float32)            # [1024]
        wqt_full = np.ascontiguousarray((Wq * ccol[:, None]).T)  # [1024(k),1024(n)]
        wkt_full = np.ascontiguousarray((Wk * ccol[:, None]).T)
        xt = np.ascontiguousarray(x[b].T).reshape(8, 128, S)
        maskb = np.ascontiguousarray(
            ((mask[b].astype(np.float32) - 1.0) * 30000.0).reshape(16, 128).T)
        for g in range(4):
            cols = slice(g * 256, (g + 1) * 256)
            in_maps.append({
                "xt": xt,
                "wqt": np.ascontiguousarray(wqt_full[:, cols]).reshape(8, 128, 256),
                "wkt": np.ascontiguousarray(wkt_full[:, cols]).reshape(8, 128, 256),
                "wvt": np.ascontiguousarray(wvt_full[:, cols]).reshape(8, 128, 256),
                "maskb": maskb,
            })
    return in_maps


def _get_nc():
    if "nc" not in _CACHE:
        _CACHE["nc"] = _build_nc()
    return _CACHE["nc"]


def kernel(x, attention_mask, Wq, Wk, Wv, **extra_kwargs):
    nc = _get_nc()
    in_maps = _host_prep(x, attention_mask, Wq, Wk, Wv)
    res = run_bass_kernel_spmd(nc, in_maps, list(range(NCORES))).results
    out = np.empty((B, S, D), dtype=np.float32)
    for c in range(NCORES):
        b, g = divmod(c, 4)
        out[b, :, g * 256:(g + 1) * 256] = res[c]["out"]
    return out



# revision 51
# speedup vs baseline: 4.9918x; 4.9918x over previous
"""Grouped self-attention (B=2, S=2048, D=1024, H=16, hd=64) on 8 trn2 cores.

Sharding: core c = b*4 + g handles batch b, heads [4g, 4g+4) (= 2 pairs of 2).

The reference's RoPE indexes its cos/sin cache by the BATCH dim and uses
neg_half = [t_first, -t_second], so rope(t)[b,s,h,d] is a pure per-(b,d)
scale that folds into rows of Wq/Wk on the host. The device kernel is then
QKV projection + softmax attention.

Performance structure (vs the f32 v1; TimelineSim 237.5us -> 180.6us):
  - whole data path in bf16 (halves DMA + SBUF traffic)
  - attention computed transposed: scores^T [k_part, q_free] -> exp on ACT
    (the global bottleneck: 128 x ~1040ns engine-busy) -> PV as
    V_aug^T @ expT with a ones-column producing the softmax denominator;
    the [65, q] result is staged bf16 and normalized/transposed on the
    host, eliminating all PE transposes and per-column scaling.
  - attention issue order is software-pipelined (act(i) | score(i+1) |
    pv(i)) so the ACT engine runs back-to-back while the PE stays a step
    ahead.
  - pair-0 K/Q projections run under the input DMA stream (K wide in the
    score PSUM ring, Q concurrently through the 1-bank projection ring);
    pair-1 K/Q and second-half V are issued before attn1 for correct RAW
    tracking but DE-prioritized below both attention loops
    (tc.high_priority with negative offset), making them a self-paced PE
    filler that packs attention's idle PE slack; work attn1 needs late
    (Q1 qh1, head-3 V) forms the tail so it lands inside attn1's window.
  - DMA count minimized (the HWDGE queue serializes ~625ns per DMA) and
    ordered by consumption: wk, xt0, wq, wv, xt1-7, mask.

Device layout per core:
  xt    [8,128,2048] bf16 = x[b].T          (k-major chunks)
  wqt   [8,128,256]  bf16 = (Wq*ceff).T cols for this core's 4 heads
  wkt   [8,128,256]  bf16
  wvt   [8,128,256]  bf16
  maskb [128,16]     f32  = (mask[b]-1)*3e4 as [S] -> [16,128].T
  outT  [260,2048]   bf16 = per local head: 64 rows of (P@V)^T + 1 denom row
"""

import numpy as np
from contextlib import ExitStack

import concourse.bass as bass
import concourse.bacc as bacc
import concourse.tile as tile
from concourse import mybir
from concourse.bass_utils import run_bass_kernel_spmd

F32 = mybir.dt.float32
BF16 = mybir.dt.bfloat16
EXP = mybir.ActivationFunctionType.Exp

B, S, D, H, HD = 2, 2048, 1024, 16, 64
NCORES = 8

_CACHE = {}


def _build_nc():
    nc = bacc.Bacc("TRN2", target_bir_lowering=False, debug=False)
    xt_d = nc.declare_dram_parameter("xt", [8, 128, S], BF16, isOutput=False)
    wqt_d = nc.declare_dram_parameter("wqt", [8, 128, 256], BF16, isOutput=False)
    wkt_d = nc.declare_dram_parameter("wkt", [8, 128, 256], BF16, isOutput=False)
    wvt_d = nc.declare_dram_parameter("wvt", [8, 128, 256], BF16, isOutput=False)
    mb_d = nc.declare_dram_parameter("maskb", [128, 16], F32, isOutput=False)
    out_d = nc.declare_dram_parameter("outT", [4 * 65, S], BF16, isOutput=True)

    with tile.TileContext(nc) as tc, ExitStack() as ctx:
        const = ctx.enter_context(tc.tile_pool(name="const", bufs=1))
        xpool = ctx.enter_context(tc.tile_pool(name="x", bufs=1))
        wpool = ctx.enter_context(tc.tile_pool(name="w", bufs=1))
        qkpool = ctx.enter_context(tc.tile_pool(name="qk", bufs=1))
        vpool = ctx.enter_context(tc.tile_pool(name="v", bufs=1))
        ep = ctx.enter_context(tc.tile_pool(name="et", bufs=3))
        ost = ctx.enter_context(tc.tile_pool(name="ost", bufs=2))
        scp = ctx.enter_context(tc.tile_pool(name="scp", bufs=2, space="PSUM"))
        pvp = ctx.enter_context(tc.tile_pool(name="pvp", bufs=1, space="PSUM"))
        pjp = ctx.enter_context(tc.tile_pool(name="pjp", bufs=2, space="PSUM"))

        # DMA order = consumption order. One DMA per weight tensor (the
        # HWDGE queue charges a large fixed overhead per DMA), xt in 8
        # chunk-tiles so projections start as soon as chunk 0 lands.
        wk = wpool.tile([128, 8, 256], BF16, tag="wk")
        wq = wpool.tile([128, 8, 256], BF16, tag="wq")
        wv = wpool.tile([128, 8, 256], BF16, tag="wv")
        xt = [xpool.tile([128, S], BF16, tag=f"xt{c}", name=f"xt{c}")
              for c in range(8)]
        mb = const.tile([128, 16], F32)
        nc.sync.dma_start(wk[:], wkt_d[:].rearrange("c p n -> p c n"))
        nc.sync.dma_start(xt[0][:], xt_d[0])
        nc.sync.dma_start(wq[:], wqt_d[:].rearrange("c p n -> p c n"))
        nc.sync.dma_start(wv[:], wvt_d[:].rearrange("c p n -> p c n"))
        for c in range(1, 8):
            nc.sync.dma_start(xt[c][:], xt_d[c])
        nc.sync.dma_start(mb[:], mb_d[:])

        qt = [qkpool.tile([128, S], BF16, tag=f"qt{p}", name=f"qt{p}") for p in range(2)]
        kt = [qkpool.tile([128, S], BF16, tag=f"kt{p}", name=f"kt{p}") for p in range(2)]
        # V in natural layout [s_part, kb, head, 64+ones]
        v_sb = vpool.tile([128, 16, 4, 65], BF16)
        nc.vector.memset(v_sb[:, :, :, 64:65], 1.0)

        def v_block(m, half):
            """One 128-row s-block of V for one head pair: PSUM [128s, 128]."""
            ps = pjp.tile([128, 128], F32, tag="pj", name=f"vps{half}_{m}")
            for kc in range(8):
                nc.tensor.matmul(
                    ps[:],
                    lhsT=xt[kc][:, m * 128:(m + 1) * 128],
                    rhs=wv[:, kc, half * 128:half * 128 + 128],
                    start=(kc == 0), stop=(kc == 7))
            nc.vector.tensor_copy(
                v_sb[:, m, 2 * half:2 * half + 2, 0:64],
                ps[:].rearrange("p (h d) -> p h d", h=2))

        def v_quarter(m, h, pool=None):
            """One 128-row s-block of V for a single head: PSUM [128s, 64]."""
            pool = pool or pjp
            ps = pool.tile([128, 64], F32, tag="pj" if pool is pjp else "sc",
                           name=f"vq{h}_{m}")
            for kc in range(8):
                nc.tensor.matmul(
                    ps[:],
                    lhsT=xt[kc][:, m * 128:(m + 1) * 128],
                    rhs=wv[:, kc, h * 64:(h + 1) * 64],
                    start=(kc == 0), stop=(kc == 7))
            nc.vector.tensor_copy(v_sb[:, m, h, 0:64], ps[:])

        def qk_group(wt, dst, pair, g, pool=None):
            """256-col s-group of a pair-`pair` Q/K projection."""
            pool = pool or pjp
            ps = pool.tile([128, 256], F32, tag="pj" if pool is pjp else "sc",
                           name=f"pj{0 if wt is wk else 1}_{pair}_{g}")
            for kc in range(8):
                nc.tensor.matmul(
                    ps[:],
                    lhsT=wt[:, kc, pair * 128:pair * 128 + 128],
                    rhs=xt[kc][:, g * 256:(g + 1) * 256],
                    start=(kc == 0), stop=(kc == 7))
            nc.vector.tensor_copy(dst[:, g * 256:(g + 1) * 256], ps[:])

        def qk_mm(ps, wt, pair, sh, kc):
            # matmul out is capped at 512 f32 per instruction (one PSUM bank)
            lo = pair * 128
            for j in range(2):
                nc.tensor.matmul(
                    ps[:, j * 512:(j + 1) * 512],
                    lhsT=wt[:, kc, lo:lo + 128],
                    rhs=xt[kc][:, sh * 1024 + j * 512: sh * 1024 + j * 512 + 512],
                    start=(kc == 0), stop=(kc == 7))

        # ---- upfront projections: K0 through the wide scp tiles and Q0
        #      through the 1-bank pjp ring concurrently (~1.1us of PE work
        #      in flight per arriving xt chunk), then pair-0 V blocks. ----
        ps_sh = [scp.tile([128, 1024], F32, tag="sc", name=f"upk_{sh}")
                 for sh in range(2)]
        q0_done = 0
        for kc in range(8):
            qk_mm(ps_sh[0], wk, 0, 0, kc)
            qk_mm(ps_sh[1], wk, 0, 1, kc)
            if q0_done < 8:
                qk_group(wq, qt[0], 0, q0_done)
                q0_done += 1
        for sh in range(2):
            nc.vector.tensor_copy(kt[0][:, sh * 1024:(sh + 1) * 1024], ps_sh[sh][:])
        for m in range(16):
            v_block(m, 0)

        # ---- attention (software-pipelined issue: act(i) | score(i+1) |
        #      pv(i) so the next score is ahead of the act-dependent pv) ----
        def attn(pair):
            flat = [(hh, qh, kb)
                    for hh in range(2) for qh in range(2) for kb in range(16)]
            sc_tiles = {}

            def score(i):
                hh, qh, kb = flat[i]
                ps = scp.tile([128, 1024], F32, tag="sc", name=f"sc{pair}_{i}")
                for j in range(2):
                    nc.tensor.matmul(
                        ps[:, j * 512:(j + 1) * 512],
                        lhsT=kt[pair][hh * 64:hh * 64 + 64, kb * 128:(kb + 1) * 128],
                        rhs=qt[pair][hh * 64:hh * 64 + 64,
                                     qh * 1024 + j * 512: qh * 1024 + j * 512 + 512],
                        start=True, stop=True)
                sc_tiles[i] = ps

            score(0)
            pv = None
            for i, (hh, qh, kb) in enumerate(flat):
                h = 2 * pair + hh
                if kb == 0:
                    pv = pvp.tile([65, 1024], F32, tag="pv", name=f"pv{pair}{hh}{qh}")
                ps = sc_tiles.pop(i)
                et = ep.tile([128, 1024], BF16, tag="et", name=f"et{pair}_{i}")
                nc.scalar.activation(et[:], ps[:], EXP,
                                     bias=mb[:, kb:kb + 1], scale=0.125)
                if i + 1 < len(flat):
                    score(i + 1)
                for j in range(2):
                    nc.tensor.matmul(
                        pv[:, j * 512:(j + 1) * 512],
                        lhsT=v_sb[:, kb, h, :],
                        rhs=et[:, j * 512:(j + 1) * 512],
                        start=(kb == 0), stop=(kb == 15))
                if kb == 15:
                    r0 = h * 65
                    if i == len(flat) - 1:
                        # last group: drain in halves so the copy/DMA of
                        # half 0 overlaps the copy of half 1 (shorter tail)
                        for j in range(2):
                            oth = ost.tile([65, 512], BF16, tag="ot",
                                           name=f"ot{pair}{hh}{qh}_{j}")
                            nc.vector.tensor_copy(oth[:], pv[:, j * 512:(j + 1) * 512])
                            nc.sync.dma_start(
                                out_d[r0:r0 + 65,
                                      qh * 1024 + j * 512: qh * 1024 + j * 512 + 512],
                                oth[:])
                    else:
                        ot = ost.tile([65, 1024], BF16, tag="ot",
                                      name=f"ot{pair}{hh}{qh}")
                        nc.vector.tensor_copy(ot[:], pv[:])
                        nc.sync.dma_start(
                            out_d[r0:r0 + 65, qh * 1024:(qh + 1) * 1024], ot[:])

        attn(0)
        # Filler work: pair-1 K/Q projections and second-half V blocks,
        # issued before attn1 (so RAW deps are tracked) but DE-prioritized
        # below BOTH attention loops via a negative high_priority offset.
        # The PE picks filler up only when attention work isn't ready, so
        # it spreads across attn0's AND attn1's PE slack; attn1's own
        # dependency stalls pull the filler forward just in time. Work
        # attn1 needs at its start comes first; work it needs late (Q1's
        # qh1 half, head-3 V) forms the tail that packs attn1's interior.
        with tc.high_priority(offset=-(1 << 20)):
            for g in range(4):
                qk_group(wq, qt[1], 1, g)
            for g in range(8):
                qk_group(wk, kt[1], 1, g)
                v_quarter(2 * g, 2)
                v_quarter(2 * g + 1, 2)
            for g in range(4, 8):
                qk_group(wq, qt[1], 1, g)
            for m in range(16):
                v_quarter(m, 3)
        attn(1)
    nc.compile()
    return nc


def _host_prep(x, attention_mask, Wq, Wk, Wv):
    bf16 = mybir.dt.np(BF16)
    x = np.asarray(x, dtype=np.float32)
    mask = np.asarray(attention_mask)
    Wq = np.asarray(Wq, dtype=np.float32)
    Wk = np.asarray(Wk, dtype=np.float32)
    Wv = np.asarray(Wv, dtype=np.float32)

    # rope fold: c_eff[b, d] = cos(b*th[d%32]) + sign(d)*sin(b*th[d%32])
    j = np.arange(0, HD, 2, dtype=np.float64) / HD          # [32]
    theta = 1.0 / (10000.0 ** j)                            # [32]
    dd = np.arange(HD)
    sign = np.where(dd < 32, 1.0, -1.0)
    in_maps = []
    wvt_full = np.ascontiguousarray(Wv.T)                   # [1024,1024]
    for b in range(B):
        ang = b * theta                                     # [32]
        ce = np.cos(ang[dd % 32]) + sign * np.sin(ang[dd % 32])  # [64]
        ccol = np.tile(ce, H).astype(np.float32)            # [1024]
        wqt_full = np.ascontiguousarray((Wq * ccol[:, None]).T)  # [1024(k),1024(n)]
        wkt_full = np.ascontiguousarray((Wk * ccol[:, None]).T)
        xtb = np.ascontiguousarray(x[b].T.astype(bf16)).reshape(8, 128, S)
        maskb = np.ascontiguousarray(
            ((mask[b].astype(np.float32) - 1.0) * 30000.0).reshape(16, 128).T)
        for g in range(4):
            cols = slice(g * 256, (g + 1) * 256)
            in_maps.append({
                "xt": xtb,
                "wqt": np.ascontiguousarray(wqt_full[:, cols].astype(bf16)).reshape(8, 128, 256),
                "wkt": np.ascontiguousarray(wkt_full[:, cols].astype(bf16)).reshape(8, 128, 256),
                "wvt": np.ascontiguousarray(wvt_full[:, cols].astype(bf16)).reshape(8, 128, 256),
                "maskb": maskb,
            })
    return in_maps


def _get_nc():
    if "nc" not in _CACHE:
        _CACHE["nc"] = _build_nc()
    return _CACHE["nc"]


def _gather(results):
    out = np.empty((B, S, D), dtype=np.float32)
    for c in range(NCORES):
        b, g = divmod(c, 4)
        o3 = np.asarray(results[c]["outT"], dtype=np.float32).reshape(4, 65, S)
        heads = o3[:, :64, :] / o3[:, 64:65, :]             # [4, 64, S]
        out[b, :, g * 256:(g + 1) * 256] = heads.transpose(2, 0, 1).reshape(S, 256)
    return out


def kernel(x, attention_mask, Wq, Wk, Wv, **extra_kwargs):
    nc = _get_nc()
    in_maps = _host_prep(x, attention_mask, Wq, Wk, Wv)
    res = run_bass_kernel_spmd(nc, in_maps, list(range(NCORES))).results
    return _gather(res)
